# revision 13
# baseline (speedup 1.0000x reference)
"""TRN2 Bass kernel for nn_BertSelfAttention_61194694034283.

Single-query attention: B=8, T=4096, H=1024, 16 heads x 64.

Algebraic restructuring (exact, up to fp rounding):
  scores[b,h,t] = q[b,h]·(Wk_h x[b,t] + bk_h)/8 = m[b,h]·x[b,t] + c[b,h]
  with m[b,h] = (q[b,h] @ Wk_h)/8.  The constant c[b,h] is uniform over t
  and drops out of the softmax.
  ctx[b,h] = Wv_h (sum_t p_t x[b,t]) + bv_h  since sum_t p_t = 1.
So the device only needs to stream x = hidden_states[:, :4096] once per
batch: scores = mT.T @ x.T (+mask), e = exp(scores) (scores are bounded,
|s| < 30, so no max-shift is needed in fp32), u = e @ x, denom = row-sums.
The tiny projections (q, m) and the final per-head Wv GEMV run on host.

Sharding: batch b -> NeuronCore b (8 cores, SPMD).
"""

import os

os.environ.setdefault("JAX_PLATFORMS", "")

import numpy as np

B, T, H, NH, HD = 8, 4096, 1024, 16, 64
TCH = 512                # tokens per device chunk
NCH = 8                  # device denom partials
MASK_NEG = -30000.0
USE_BF16 = True          # bf16 device compute (fp32r pipeline if False)

_prog_cache = {}
LAST_RESULTS = None


def _build_bf16():
    import concourse.tile as tile
    from concourse import bacc, mybir
    from contextlib import ExitStack

    F32 = mybir.dt.float32
    BF16 = mybir.dt.bfloat16
    TC = 512             # tokens per pipeline chunk
    NC_ = T // TC        # 8 chunks
    NJ = TC // 128       # 4 token-tiles per chunk
    KC = H // 128        # 8 contraction tiles

    nc = bacc.Bacc("TRN2", target_bir_lowering=False, debug=False)
    X = nc.dram_tensor("X", [T, H], F32, kind="ExternalInput").ap()
    MT = nc.dram_tensor("MT", [H, NH], BF16, kind="ExternalInput").ap()
    AUX = nc.dram_tensor("AUX", [1, 128 + T], BF16, kind="ExternalInput").ap()
    IDENT = nc.dram_tensor("IDENT", [128, 128], BF16, kind="ExternalInput").ap()
    U = nc.dram_tensor("U", [NH, H], F32, kind="ExternalOutput").ap()
    D = nc.dram_tensor("D", [NH, NC_], F32, kind="ExternalOutput").ap()

    with tile.TileContext(nc) as tc, ExitStack() as ctx:
        singles = ctx.enter_context(tc.tile_pool(name="singles", bufs=1))
        xpool = ctx.enter_context(tc.tile_pool(name="xpool", bufs=5))
        stpool = ctx.enter_context(tc.tile_pool(name="stpool", bufs=4))
        xtpool = ctx.enter_context(tc.tile_pool(name="xtpool", bufs=5))
        epool = ctx.enter_context(tc.tile_pool(name="epool", bufs=2))
        ptpool = ctx.enter_context(tc.tile_pool(name="ptpool", bufs=2))
        ps_xt = ctx.enter_context(tc.tile_pool(name="ps_xt", bufs=4, space="PSUM"))
        ps_sc = ctx.enter_context(tc.tile_pool(name="ps_sc", bufs=1, space="PSUM"))
        ps_pt = ctx.enter_context(tc.tile_pool(name="ps_pt", bufs=1, space="PSUM"))
        ps_u = ctx.enter_context(tc.tile_pool(name="ps_u", bufs=1, space="PSUM"))

        def load_chunk(c):
            xt_ = xpool.tile([128, NJ, H], BF16, tag="x")
            src_ = X[c * TC : (c + 1) * TC, :].rearrange("(j p) h -> p j h", p=128)
            nc.gpsimd.dma_start(out=xt_, in_=src_)
            return xt_

        def load_chunk0_fast():
            # HWDGE fp32 staging + engine casts: skips the SWDGE init stall
            xt_ = xpool.tile([128, NJ, H], BF16, tag="x")
            src_ = X[0:TC, :].rearrange("(j p) h -> p j h", p=128)
            for j in range(NJ):
                st_ = stpool.tile([128, H], F32, tag="st")
                nc.sync.dma_start(out=st_, in_=src_[:, j, :])
                if j % 2 == 0:
                    nc.vector.tensor_copy(xt_[:, j, :], st_)
                else:
                    nc.scalar.copy(xt_[:, j, :], st_)
            return xt_

        x_tiles = [load_chunk0_fast()]

        ident = singles.tile([128, 128], BF16)
        nc.sync.dma_start(out=ident, in_=IDENT)
        aux_sb = singles.tile([1, 128 + T], BF16)
        nc.sync.dma_start(out=aux_sb, in_=AUX)
        ones_row = aux_sb[:, 0:NH]
        mT_sb = singles.tile([128, KC, NH], BF16)
        nc.sync.dma_start(out=mT_sb, in_=MT.rearrange("(c p) n -> p c n", p=128))

        d_sb = singles.tile([NH, NC_], F32)
        u_ps = ps_u.tile([NH, H], F32)
        e_tiles = {}

        x_tiles.append(load_chunk(1))
        x_tiles.append(load_chunk(2))

        # PE warm-up: dummy matmuls on the identity while chunk-0 DMA lands.
        # Keeps the HAM activity monitor busy so real matmuls start at 2.4GHz.
        warm_ps = ps_pt.tile([128, 128], F32, tag="pt")
        for _ in range(56):
            nc.tensor.matmul(warm_ps, ident, ident, start=True, stop=True)

        def emit_tail(c):
            # probsT + u-accumulation for chunk c
            x_c = x_tiles[c]
            e_sb = e_tiles[c]
            pT_ps = ps_pt.tile([128, NJ, NH], BF16, tag="pt")
            for j in range(NJ):
                nc.tensor.transpose(
                    pT_ps[:, j, :],
                    e_sb[:, j * 128 : (j + 1) * 128],
                    ident[0:NH, 0:NH],
                )
            pT_sb = ptpool.tile([128, NJ, NH], BF16, tag="pts")
            nc.vector.tensor_copy(pT_sb, pT_ps)
            for j in range(NJ):
                for n in range(2):
                    nc.tensor.matmul(
                        u_ps[:, n * 512 : (n + 1) * 512],
                        pT_sb[:, j, :],
                        x_c[:, j, n * 512 : (n + 1) * 512],
                        start=(c == 0 and j == 0),
                        stop=(c == NC_ - 1 and j == NJ - 1),
                    )

        for c in range(NC_):
            if c + 3 < NC_:
                x_tiles.append(load_chunk(c + 3))
            x_c = x_tiles[c]

            sc_ps = ps_sc.tile([NH, TC], F32, tag="sc")
            xT_done = []
            for kc2 in range(0, KC, 2):
                for kc in (kc2, kc2 + 1):
                    xT_ps = ps_xt.tile([128, TC], BF16, tag="xt")
                    for j in range(NJ):
                        nc.tensor.transpose(
                            xT_ps[:, j * 128 : (j + 1) * 128],
                            x_c[:, j, kc * 128 : (kc + 1) * 128],
                            ident,
                        )
                    xT_sb = xtpool.tile([128, TC], BF16, tag="xts")
                    nc.vector.tensor_copy(xT_sb, xT_ps)
                    xT_done.append(xT_sb)
                if kc2 == 0:
                    # open the accumulation group with the mask rank-1 add
                    # (no copy dependency - fills the first bubble)
                    nc.tensor.matmul(
                        sc_ps, ones_row,
                        aux_sb[:, 128 + c * TC : 128 + (c + 1) * TC],
                        start=True, stop=False,
                    )
                if kc2 >= 2:
                    # paired mm1s, lag one pair: copy latency hidden
                    nc.tensor.matmul(
                        sc_ps, mT_sb[:, kc2 - 2, :], xT_done[kc2 - 2],
                        start=False, stop=False,
                    )
                    nc.tensor.matmul(
                        sc_ps, mT_sb[:, kc2 - 1, :], xT_done[kc2 - 1],
                        start=False, stop=False,
                    )
                if kc2 == 4 and c >= 1:
                    emit_tail(c - 1)
            nc.tensor.matmul(
                sc_ps, mT_sb[:, KC - 2, :], xT_done[KC - 2],
                start=False, stop=False,
            )
            nc.tensor.matmul(
                sc_ps, mT_sb[:, KC - 1, :], xT_done[KC - 1],
                start=False, stop=True,
            )

            e_sb = epool.tile([NH, TC], BF16, tag="e")
            e_tiles[c] = e_sb
            nc.scalar.activation(
                out=e_sb, in_=sc_ps,
                func=mybir.ActivationFunctionType.Exp,
                accum_out=d_sb[:, c : c + 1],
            )
        nc.sync.dma_start(out=D, in_=d_sb)
        emit_tail(NC_ - 1)

        u_sb = singles.tile([NH, H], F32)
        for n in range(2):
            nc.vector.tensor_copy(
                u_sb[:, n * 512 : (n + 1) * 512], u_ps[:, n * 512 : (n + 1) * 512]
            )
            nc.sync.dma_start(
                out=U[:, n * 512 : (n + 1) * 512],
                in_=u_sb[:, n * 512 : (n + 1) * 512],
            )

    nc.compile()
    return nc


def _build_f32r():
    import concourse.tile as tile
    from concourse import bacc, mybir
    from contextlib import ExitStack

    F32 = mybir.dt.float32
    F32R = mybir.dt.float32r

    nc = bacc.Bacc("TRN2", target_bir_lowering=False, debug=False)
    X = nc.dram_tensor("X", [T, H], F32, kind="ExternalInput").ap()
    MT = nc.dram_tensor("MT", [H, 128], F32, kind="ExternalInput").ap()
    AUX = nc.dram_tensor("AUX", [1, 128 + T], F32, kind="ExternalInput").ap()
    IDENT = nc.dram_tensor("IDENT", [128, 128], F32, kind="ExternalInput").ap()
    U = nc.dram_tensor("U", [NH, H], F32, kind="ExternalOutput").ap()
    D = nc.dram_tensor("D", [NH, NCH], F32, kind="ExternalOutput").ap()

    with tile.TileContext(nc) as tc, ExitStack() as ctx:
        singles = ctx.enter_context(tc.tile_pool(name="singles", bufs=1))
        xpool = ctx.enter_context(tc.tile_pool(name="xpool", bufs=5))
        stpool = ctx.enter_context(tc.tile_pool(name="stpool", bufs=4))
        xtpool = ctx.enter_context(tc.tile_pool(name="xtpool", bufs=3))
        epool = ctx.enter_context(tc.tile_pool(name="epool", bufs=2))
        ps_xt = ctx.enter_context(tc.tile_pool(name="ps_xt", bufs=2, space="PSUM"))
        ps_sc = ctx.enter_context(tc.tile_pool(name="ps_sc", bufs=1, space="PSUM"))
        ps_pt = ctx.enter_context(tc.tile_pool(name="ps_pt", bufs=2, space="PSUM"))
        ps_u = ctx.enter_context(tc.tile_pool(name="ps_u", bufs=1, space="PSUM"))

        ident = singles.tile([128, 128], F32R)
        nc.gpsimd.dma_start(out=ident, in_=IDENT)
        aux_sb = singles.tile([1, 128 + T], F32R)
        nc.gpsimd.dma_start(out=aux_sb, in_=AUX)
        ones_row = aux_sb[:, 0:128]
        mT_sb = singles.tile([128, H // 128, 128], F32R)
        nc.gpsimd.dma_start(out=mT_sb, in_=MT.rearrange("(c p) n -> p c n", p=128))

        d_sb = singles.tile([NH, NCH], F32)
        u_ps = ps_u.tile([128, H], F32)

        pT_hold = []
        for i in range(2):
            t_ = singles.tile([128, 4, 128], F32R, tag=f"pT{i}")
            nc.vector.memset(t_.bitcast(F32), 0.0)
            pT_hold.append(t_)

        for c in range(NCH):
            x_c = xpool.tile([128, 4, H], F32R, tag="x")
            nc.gpsimd.dma_start(
                out=x_c,
                in_=X[c * TCH : (c + 1) * TCH, :].rearrange(
                    "(j p) h -> p j h", p=128
                ),
            )

            sc_ps = ps_sc.tile([128, TCH], F32, tag="sc")
            for kc in range(H // 128):
                xT_ps = ps_xt.tile([128, TCH], F32, tag="xt")
                for j in range(4):
                    nc.tensor.transpose(
                        xT_ps.bitcast(F32R)[:, j * 128 : (j + 1) * 128],
                        x_c[:, j, kc * 128 : (kc + 1) * 128],
                        ident,
                    )
                xT_sb = xtpool.tile([128, TCH], F32R, tag="xts")
                if kc % 2 == 0:
                    nc.vector.tensor_copy(xT_sb, xT_ps)
                else:
                    nc.scalar.copy(xT_sb, xT_ps)
                nc.tensor.matmul(
                    sc_ps, mT_sb[:, kc, :], xT_sb,
                    start=(kc == 0), stop=False,
                )
            nc.tensor.matmul(
                sc_ps, ones_row,
                aux_sb[:, 128 + c * TCH : 128 + (c + 1) * TCH],
                start=False, stop=True,
            )

            e_sb = epool.tile([NH, TCH], F32R, tag="e")
            nc.scalar.activation(
                out=e_sb, in_=sc_ps[0:NH, :],
                func=mybir.ActivationFunctionType.Exp,
                accum_out=d_sb[:, c : c + 1],
            )

            pT_ps = ps_pt.tile([128, 4, NH], F32, tag="pt")
            for j in range(4):
                nc.tensor.transpose(
                    pT_ps.bitcast(F32R)[:, j, :],
                    e_sb[:, j * 128 : (j + 1) * 128],
                    ident[0:NH, 0:NH],
                )
            hold = pT_hold[c % 2]
            nc.vector.tensor_copy(hold[:, :, 0:NH], pT_ps)

            for j in range(4):
                for n in range(2):
                    nc.tensor.matmul(
                        u_ps[:, n * TCH : (n + 1) * TCH],
                        hold[:, j, :],
                        x_c[:, j, n * TCH : (n + 1) * TCH],
                        start=(c == 0 and j == 0),
                        stop=(c == NCH - 1 and j == 3),
                    )

        u_sb = singles.tile([NH, H], F32)
        nc.vector.tensor_copy(u_sb, u_ps[0:NH, :])
        nc.sync.dma_start(out=U, in_=u_sb)
        nc.sync.dma_start(out=D, in_=d_sb)

    nc.compile()
    return nc


def kernel(
    hidden_states, cell_state, Wq, bq, Wk, bk, Wv, bv, attention_mask, t
):
    global LAST_RESULTS
    import ml_dtypes
    from concourse.bass_utils import run_bass_kernel_spmd

    hs = np.asarray(hidden_states, dtype=np.float32)
    cell = np.asarray(cell_state, dtype=np.float32)
    Wq_ = np.asarray(Wq, dtype=np.float32)
    bq_ = np.asarray(bq, dtype=np.float32)
    Wk_ = np.asarray(Wk, dtype=np.float32)
    bk_ = np.asarray(bk, dtype=np.float32)  # noqa: F841 (drops out of softmax)
    Wv_ = np.asarray(Wv, dtype=np.float32)
    bv_ = np.asarray(bv, dtype=np.float32)
    mask = np.asarray(attention_mask)
    tt = int(t)
    assert tt == T and hs.shape == (B, T + 1, H)

    # host precompute: q, m (fold the 1/sqrt(hd) scale into m)
    q = (cell @ Wq_.T + bq_).reshape(B, NH, HD)
    Wk_h = Wk_.reshape(NH, HD, H)
    m = np.einsum("bhd,hdi->bhi", q, Wk_h) / np.sqrt(HD)  # [B, NH, H]

    maskadd = np.where(mask[:, :T] == 0, MASK_NEG, 0.0).astype(np.float32)

    if USE_BF16:
        bf16 = ml_dtypes.bfloat16
        mT = np.ascontiguousarray(
            m.transpose(0, 2, 1)
        ).astype(bf16)  # [B, H, NH]
        aux = np.zeros((B, 1, 128 + T), dtype=bf16)
        aux[:, 0, :NH] = bf16(1.0)
        aux[:, 0, 128:] = maskadd.astype(bf16)
        eye = np.eye(128, dtype=np.float32).astype(bf16)
    else:
        mT = np.zeros((B, H, 128), dtype=np.float32)
        mT[:, :, :NH] = m.transpose(0, 2, 1)
        aux = np.zeros((B, 1, 128 + T), dtype=np.float32)
        aux[:, 0, :128] = 1.0
        aux[:, 0, 128:] = maskadd
        eye = np.eye(128, dtype=np.float32)

    key = "bf16" if USE_BF16 else "f32r"
    if key not in _prog_cache:
        _prog_cache[key] = _build_bf16() if USE_BF16 else _build_f32r()
    nc = _prog_cache[key]

    in_maps = [
        {
            "X": np.ascontiguousarray(hs[b, :T, :]),
            "MT": mT[b],
            "AUX": aux[b],
            "IDENT": eye,
        }
        for b in range(B)
    ]
    res = run_bass_kernel_spmd(nc, in_maps, core_ids=list(range(B)))
    LAST_RESULTS = res

    U = np.stack(
        [np.asarray(res.results[b]["U"], dtype=np.float32) for b in range(B)]
    )
    Dp = np.stack(
        [np.asarray(res.results[b]["D"], dtype=np.float32) for b in range(B)]
    )
    denom = Dp.sum(axis=2)  # [B, NH]
    u = U / denom[:, :, None]
    Wv_h = Wv_.reshape(NH, HD, H)
    ctx = np.einsum("bhi,hdi->bhd", u, Wv_h) + bv_.reshape(NH, HD)[None]
    return ctx.reshape(B, NH * HD).astype(np.float32)


# revision 14
# speedup vs baseline: 1.0655x; 1.0655x over previous
"""TRN2 Bass kernel for nn_BertSelfAttention_61194694034283.

Single-query attention: B=8, T=4096, H=1024, 16 heads x 64.

Algebraic restructuring (exact, up to fp rounding):
  scores[b,h,t] = q[b,h]·(Wk_h x[b,t] + bk_h)/8 = m[b,h]·x[b,t] + c[b,h]
  with m[b,h] = (q[b,h] @ Wk_h)/8.  The constant c[b,h] is uniform over t
  and drops out of the softmax.
  ctx[b,h] = Wv_h (sum_t p_t x[b,t]) + bv_h  since sum_t p_t = 1.
So the device only needs to stream x = hidden_states[:, :4096] once per
batch: scores = mT.T @ x.T (+mask), e = exp(scores) (scores are bounded,
|s| < 30, so no max-shift is needed in fp32), u = e @ x, denom = row-sums.
The tiny projections (q, m) and the final per-head Wv GEMV run on host.

Sharding: batch b -> NeuronCore b (8 cores, SPMD).
"""

import os

os.environ.setdefault("JAX_PLATFORMS", "")

import numpy as np

B, T, H, NH, HD = 8, 4096, 1024, 16, 64
TCH = 512                # tokens per device chunk
NCH = 8                  # device denom partials
MASK_NEG = -30000.0
USE_BF16 = True          # bf16 device compute (fp32r pipeline if False)

_prog_cache = {}
LAST_RESULTS = None


def _build_bf16():
    import concourse.tile as tile
    from concourse import bacc, mybir
    from contextlib import ExitStack

    F32 = mybir.dt.float32
    BF16 = mybir.dt.bfloat16
    TC = 512             # tokens per pipeline chunk
    NC_ = T // TC        # 8 chunks
    NJ = TC // 128       # 4 token-tiles per chunk
    KC = H // 128        # 8 contraction tiles

    nc = bacc.Bacc("TRN2", target_bir_lowering=False, debug=False)
    X = nc.dram_tensor("X", [T, H], F32, kind="ExternalInput").ap()
    MT = nc.dram_tensor("MT", [H, NH], BF16, kind="ExternalInput").ap()
    AUX = nc.dram_tensor("AUX", [1, 128 + T], BF16, kind="ExternalInput").ap()
    IDENT = nc.dram_tensor("IDENT", [128, 128], BF16, kind="ExternalInput").ap()
    U = nc.dram_tensor("U", [NH, H], F32, kind="ExternalOutput").ap()
    D = nc.dram_tensor("D", [NH, NC_], F32, kind="ExternalOutput").ap()

    with tile.TileContext(nc) as tc, ExitStack() as ctx:
        singles = ctx.enter_context(tc.tile_pool(name="singles", bufs=1))
        xpool = ctx.enter_context(tc.tile_pool(name="xpool", bufs=5))
        stpool = ctx.enter_context(tc.tile_pool(name="stpool", bufs=4))
        xtpool = ctx.enter_context(tc.tile_pool(name="xtpool", bufs=5))
        epool = ctx.enter_context(tc.tile_pool(name="epool", bufs=2))
        ptpool = ctx.enter_context(tc.tile_pool(name="ptpool", bufs=2))
        ps_xt = ctx.enter_context(tc.tile_pool(name="ps_xt", bufs=4, space="PSUM"))
        ps_sc = ctx.enter_context(tc.tile_pool(name="ps_sc", bufs=1, space="PSUM"))
        ps_pt = ctx.enter_context(tc.tile_pool(name="ps_pt", bufs=1, space="PSUM"))
        ps_u = ctx.enter_context(tc.tile_pool(name="ps_u", bufs=1, space="PSUM"))

        def load_chunk(c):
            xt_ = xpool.tile([128, NJ, H], BF16, tag="x")
            src_ = X[c * TC : (c + 1) * TC, :].rearrange("(j p) h -> p j h", p=128)
            nc.gpsimd.dma_start(out=xt_, in_=src_)
            return xt_

        def load_chunk0_split():
            xt_ = xpool.tile([128, NJ, H], BF16, tag="x")
            src_ = X[0:TC, :].rearrange("(j p) h -> p j h", p=128)
            for j in range(NJ):
                nc.gpsimd.dma_start(out=xt_[:, j, :], in_=src_[:, j, :])
            return xt_

        x_tiles = [load_chunk0_split()]

        ident = singles.tile([128, 128], BF16)
        nc.sync.dma_start(out=ident, in_=IDENT)
        aux_sb = singles.tile([1, 128 + T], BF16)
        nc.sync.dma_start(out=aux_sb, in_=AUX)
        ones_row = aux_sb[:, 0:NH]
        mT_sb = singles.tile([128, KC, NH], BF16)
        nc.sync.dma_start(out=mT_sb, in_=MT.rearrange("(c p) n -> p c n", p=128))

        d_sb = singles.tile([NH, NC_], F32)
        u_ps = ps_u.tile([NH, H], F32)
        e_tiles = {}

        x_tiles.append(load_chunk(1))

        # PE warm-up: dummy matmuls on the identity while chunk-0 DMA lands.
        # Keeps the HAM activity monitor busy so real matmuls start at 2.4GHz.
        warm_ps = ps_pt.tile([128, 128], F32, tag="pt")
        for _ in range(56):
            nc.tensor.matmul(warm_ps, ident, ident, start=True, stop=True)

        def emit_tail(c):
            # probsT + u-accumulation for chunk c
            x_c = x_tiles[c]
            e_sb = e_tiles[c]
            pT_ps = ps_pt.tile([128, NJ, NH], BF16, tag="pt")
            for j in range(NJ):
                nc.tensor.transpose(
                    pT_ps[:, j, :],
                    e_sb[:, j * 128 : (j + 1) * 128],
                    ident[0:NH, 0:NH],
                )
            pT_sb = ptpool.tile([128, NJ, NH], BF16, tag="pts")
            nc.vector.tensor_copy(pT_sb, pT_ps)
            for j in range(NJ):
                for n in range(2):
                    nc.tensor.matmul(
                        u_ps[:, n * 512 : (n + 1) * 512],
                        pT_sb[:, j, :],
                        x_c[:, j, n * 512 : (n + 1) * 512],
                        start=(c == 0 and j == 0),
                        stop=(c == NC_ - 1 and j == NJ - 1),
                    )

        for c in range(NC_):
            if c + 2 < NC_:
                x_tiles.append(load_chunk(c + 2))
            x_c = x_tiles[c]

            sc_ps = ps_sc.tile([NH, TC], F32, tag="sc")
            xT_done = []
            for kc2 in range(0, KC, 2):
                for kc in (kc2, kc2 + 1):
                    xT_ps = ps_xt.tile([128, TC], BF16, tag="xt")
                    for j in range(NJ):
                        nc.tensor.transpose(
                            xT_ps[:, j * 128 : (j + 1) * 128],
                            x_c[:, j, kc * 128 : (kc + 1) * 128],
                            ident,
                        )
                    xT_sb = xtpool.tile([128, TC], BF16, tag="xts")
                    nc.vector.tensor_copy(xT_sb, xT_ps)
                    xT_done.append(xT_sb)
                if kc2 == 0:
                    # open the accumulation group with the mask rank-1 add
                    # (no copy dependency - fills the first bubble)
                    nc.tensor.matmul(
                        sc_ps, ones_row,
                        aux_sb[:, 128 + c * TC : 128 + (c + 1) * TC],
                        start=True, stop=False,
                    )
                if kc2 >= 2:
                    # paired mm1s, lag one pair: copy latency hidden
                    nc.tensor.matmul(
                        sc_ps, mT_sb[:, kc2 - 2, :], xT_done[kc2 - 2],
                        start=False, stop=False,
                    )
                    nc.tensor.matmul(
                        sc_ps, mT_sb[:, kc2 - 1, :], xT_done[kc2 - 1],
                        start=False, stop=False,
                    )
                if kc2 == 4 and c >= 1:
                    emit_tail(c - 1)
            nc.tensor.matmul(
                sc_ps, mT_sb[:, KC - 2, :], xT_done[KC - 2],
                start=False, stop=False,
            )
            nc.tensor.matmul(
                sc_ps, mT_sb[:, KC - 1, :], xT_done[KC - 1],
                start=False, stop=True,
            )

            e_sb = epool.tile([NH, TC], BF16, tag="e")
            e_tiles[c] = e_sb
            nc.scalar.activation(
                out=e_sb, in_=sc_ps,
                func=mybir.ActivationFunctionType.Exp,
                accum_out=d_sb[:, c : c + 1],
            )
        nc.sync.dma_start(out=D, in_=d_sb)
        emit_tail(NC_ - 1)

        u_sb = singles.tile([NH, H], F32)
        for n in range(2):
            nc.vector.tensor_copy(
                u_sb[:, n * 512 : (n + 1) * 512], u_ps[:, n * 512 : (n + 1) * 512]
            )
            nc.sync.dma_start(
                out=U[:, n * 512 : (n + 1) * 512],
                in_=u_sb[:, n * 512 : (n + 1) * 512],
            )

    nc.compile()
    return nc


def _build_f32r():
    import concourse.tile as tile
    from concourse import bacc, mybir
    from contextlib import ExitStack

    F32 = mybir.dt.float32
    F32R = mybir.dt.float32r

    nc = bacc.Bacc("TRN2", target_bir_lowering=False, debug=False)
    X = nc.dram_tensor("X", [T, H], F32, kind="ExternalInput").ap()
    MT = nc.dram_tensor("MT", [H, 128], F32, kind="ExternalInput").ap()
    AUX = nc.dram_tensor("AUX", [1, 128 + T], F32, kind="ExternalInput").ap()
    IDENT = nc.dram_tensor("IDENT", [128, 128], F32, kind="ExternalInput").ap()
    U = nc.dram_tensor("U", [NH, H], F32, kind="ExternalOutput").ap()
    D = nc.dram_tensor("D", [NH, NCH], F32, kind="ExternalOutput").ap()

    with tile.TileContext(nc) as tc, ExitStack() as ctx:
        singles = ctx.enter_context(tc.tile_pool(name="singles", bufs=1))
        xpool = ctx.enter_context(tc.tile_pool(name="xpool", bufs=5))
        stpool = ctx.enter_context(tc.tile_pool(name="stpool", bufs=4))
        xtpool = ctx.enter_context(tc.tile_pool(name="xtpool", bufs=3))
        epool = ctx.enter_context(tc.tile_pool(name="epool", bufs=2))
        ps_xt = ctx.enter_context(tc.tile_pool(name="ps_xt", bufs=2, space="PSUM"))
        ps_sc = ctx.enter_context(tc.tile_pool(name="ps_sc", bufs=1, space="PSUM"))
        ps_pt = ctx.enter_context(tc.tile_pool(name="ps_pt", bufs=2, space="PSUM"))
        ps_u = ctx.enter_context(tc.tile_pool(name="ps_u", bufs=1, space="PSUM"))

        ident = singles.tile([128, 128], F32R)
        nc.gpsimd.dma_start(out=ident, in_=IDENT)
        aux_sb = singles.tile([1, 128 + T], F32R)
        nc.gpsimd.dma_start(out=aux_sb, in_=AUX)
        ones_row = aux_sb[:, 0:128]
        mT_sb = singles.tile([128, H // 128, 128], F32R)
        nc.gpsimd.dma_start(out=mT_sb, in_=MT.rearrange("(c p) n -> p c n", p=128))

        d_sb = singles.tile([NH, NCH], F32)
        u_ps = ps_u.tile([128, H], F32)

        pT_hold = []
        for i in range(2):
            t_ = singles.tile([128, 4, 128], F32R, tag=f"pT{i}")
            nc.vector.memset(t_.bitcast(F32), 0.0)
            pT_hold.append(t_)

        for c in range(NCH):
            x_c = xpool.tile([128, 4, H], F32R, tag="x")
            nc.gpsimd.dma_start(
                out=x_c,
                in_=X[c * TCH : (c + 1) * TCH, :].rearrange(
                    "(j p) h -> p j h", p=128
                ),
            )

            sc_ps = ps_sc.tile([128, TCH], F32, tag="sc")
            for kc in range(H // 128):
                xT_ps = ps_xt.tile([128, TCH], F32, tag="xt")
                for j in range(4):
                    nc.tensor.transpose(
                        xT_ps.bitcast(F32R)[:, j * 128 : (j + 1) * 128],
                        x_c[:, j, kc * 128 : (kc + 1) * 128],
                        ident,
                    )
                xT_sb = xtpool.tile([128, TCH], F32R, tag="xts")
                if kc % 2 == 0:
                    nc.vector.tensor_copy(xT_sb, xT_ps)
                else:
                    nc.scalar.copy(xT_sb, xT_ps)
                nc.tensor.matmul(
                    sc_ps, mT_sb[:, kc, :], xT_sb,
                    start=(kc == 0), stop=False,
                )
            nc.tensor.matmul(
                sc_ps, ones_row,
                aux_sb[:, 128 + c * TCH : 128 + (c + 1) * TCH],
                start=False, stop=True,
            )

            e_sb = epool.tile([NH, TCH], F32R, tag="e")
            nc.scalar.activation(
                out=e_sb, in_=sc_ps[0:NH, :],
                func=mybir.ActivationFunctionType.Exp,
                accum_out=d_sb[:, c : c + 1],
            )

            pT_ps = ps_pt.tile([128, 4, NH], F32, tag="pt")
            for j in range(4):
                nc.tensor.transpose(
                    pT_ps.bitcast(F32R)[:, j, :],
                    e_sb[:, j * 128 : (j + 1) * 128],
                    ident[0:NH, 0:NH],
                )
            hold = pT_hold[c % 2]
            nc.vector.tensor_copy(hold[:, :, 0:NH], pT_ps)

            for j in range(4):
                for n in range(2):
                    nc.tensor.matmul(
                        u_ps[:, n * TCH : (n + 1) * TCH],
                        hold[:, j, :],
                        x_c[:, j, n * TCH : (n + 1) * TCH],
                        start=(c == 0 and j == 0),
                        stop=(c == NCH - 1 and j == 3),
                    )

        u_sb = singles.tile([NH, H], F32)
        nc.vector.tensor_copy(u_sb, u_ps[0:NH, :])
        nc.sync.dma_start(out=U, in_=u_sb)
        nc.sync.dma_start(out=D, in_=d_sb)

    nc.compile()
    return nc


def kernel(
    hidden_states, cell_state, Wq, bq, Wk, bk, Wv, bv, attention_mask, t
):
    global LAST_RESULTS
    import ml_dtypes
    from concourse.bass_utils import run_bass_kernel_spmd

    hs = np.asarray(hidden_states, dtype=np.float32)
    cell = np.asarray(cell_state, dtype=np.float32)
    Wq_ = np.asarray(Wq, dtype=np.float32)
    bq_ = np.asarray(bq, dtype=np.float32)
    Wk_ = np.asarray(Wk, dtype=np.float32)
    bk_ = np.asarray(bk, dtype=np.float32)  # noqa: F841 (drops out of softmax)
    Wv_ = np.asarray(Wv, dtype=np.float32)
    bv_ = np.asarray(bv, dtype=np.float32)
    mask = np.asarray(attention_mask)
    tt = int(t)
    assert tt == T and hs.shape == (B, T + 1, H)

    # host precompute: q, m (fold the 1/sqrt(hd) scale into m)
    q = (cell @ Wq_.T + bq_).reshape(B, NH, HD)
    Wk_h = Wk_.reshape(NH, HD, H)
    m = np.einsum("bhd,hdi->bhi", q, Wk_h) / np.sqrt(HD)  # [B, NH, H]

    maskadd = np.where(mask[:, :T] == 0, MASK_NEG, 0.0).astype(np.float32)

    if USE_BF16:
        bf16 = ml_dtypes.bfloat16
        mT = np.ascontiguousarray(
            m.transpose(0, 2, 1)
        ).astype(bf16)  # [B, H, NH]
        aux = np.zeros((B, 1, 128 + T), dtype=bf16)
        aux[:, 0, :NH] = bf16(1.0)
        aux[:, 0, 128:] = maskadd.astype(bf16)
        eye = np.eye(128, dtype=np.float32).astype(bf16)
    else:
        mT = np.zeros((B, H, 128), dtype=np.float32)
        mT[:, :, :NH] = m.transpose(0, 2, 1)
        aux = np.zeros((B, 1, 128 + T), dtype=np.float32)
        aux[:, 0, :128] = 1.0
        aux[:, 0, 128:] = maskadd
        eye = np.eye(128, dtype=np.float32)

    key = "bf16" if USE_BF16 else "f32r"
    if key not in _prog_cache:
        _prog_cache[key] = _build_bf16() if USE_BF16 else _build_f32r()
    nc = _prog_cache[key]

    in_maps = [
        {
            "X": np.ascontiguousarray(hs[b, :T, :]),
            "MT": mT[b],
            "AUX": aux[b],
            "IDENT": eye,
        }
        for b in range(B)
    ]
    res = run_bass_kernel_spmd(nc, in_maps, core_ids=list(range(B)))
    LAST_RESULTS = res

    U = np.stack(
        [np.asarray(res.results[b]["U"], dtype=np.float32) for b in range(B)]
    )
    Dp = np.stack(
        [np.asarray(res.results[b]["D"], dtype=np.float32) for b in range(B)]
    )
    denom = Dp.sum(axis=2)  # [B, NH]
    u = U / denom[:, :, None]
    Wv_h = Wv_.reshape(NH, HD, H)
    ctx = np.einsum("bhi,hdi->bhd", u, Wv_h) + bv_.reshape(NH, HD)[None]
    return ctx.reshape(B, NH * HD).astype(np.float32)


# revision 15
# speedup vs baseline: 1.1645x; 1.0930x over previous
"""TRN2 Bass kernel for nn_BertSelfAttention_61194694034283.

Single-query attention: B=8, T=4096, H=1024, 16 heads x 64.

Algebraic restructuring (exact, up to fp rounding):
  scores[b,h,t] = q[b,h]·(Wk_h x[b,t] + bk_h)/8 = m[b,h]·x[b,t] + c[b,h]
  with m[b,h] = (q[b,h] @ Wk_h)/8.  The constant c[b,h] is uniform over t
  and drops out of the softmax.
  ctx[b,h] = Wv_h (sum_t p_t x[b,t]) + bv_h  since sum_t p_t = 1.
So the device only needs to stream x = hidden_states[:, :4096] once per
batch: scores = mT.T @ x.T (+mask), e = exp(scores) (scores are bounded,
|s| < 30, so no max-shift is needed in fp32), u = e @ x, denom = row-sums.
The tiny projections (q, m) and the final per-head Wv GEMV run on host.

Sharding: batch b -> NeuronCore b (8 cores, SPMD).
"""

import os

os.environ.setdefault("JAX_PLATFORMS", "")

import numpy as np

B, T, H, NH, HD = 8, 4096, 1024, 16, 64
TCH = 512                # tokens per device chunk
NCH = 8                  # device denom partials
MASK_NEG = -30000.0
USE_BF16 = True          # bf16 device compute (fp32r pipeline if False)

_prog_cache = {}
LAST_RESULTS = None


def _build_bf16():
    import concourse.tile as tile
    from concourse import bacc, mybir
    from contextlib import ExitStack

    F32 = mybir.dt.float32
    BF16 = mybir.dt.bfloat16
    TC = 512             # tokens per pipeline chunk
    NC_ = T // TC        # 8 chunks
    NJ = TC // 128       # 4 token-tiles per chunk
    KC = H // 128        # 8 contraction tiles

    nc = bacc.Bacc("TRN2", target_bir_lowering=False, debug=False)
    X = nc.dram_tensor("X", [T, H], F32, kind="ExternalInput").ap()
    MT = nc.dram_tensor("MT", [H, NH], BF16, kind="ExternalInput").ap()
    AUX = nc.dram_tensor("AUX", [1, 128 + T], BF16, kind="ExternalInput").ap()
    IDENT = nc.dram_tensor("IDENT", [128, 128], BF16, kind="ExternalInput").ap()
    U = nc.dram_tensor("U", [NH, H], F32, kind="ExternalOutput").ap()
    D = nc.dram_tensor("D", [NH, NC_], F32, kind="ExternalOutput").ap()

    with tile.TileContext(nc) as tc, ExitStack() as ctx:
        singles = ctx.enter_context(tc.tile_pool(name="singles", bufs=1))
        xpool = ctx.enter_context(tc.tile_pool(name="xpool", bufs=5))
        stpool = ctx.enter_context(tc.tile_pool(name="stpool", bufs=4))
        xtpool = ctx.enter_context(tc.tile_pool(name="xtpool", bufs=5))
        epool = ctx.enter_context(tc.tile_pool(name="epool", bufs=2))
        ptpool = ctx.enter_context(tc.tile_pool(name="ptpool", bufs=2))
        ps_xt = ctx.enter_context(tc.tile_pool(name="ps_xt", bufs=3, space="PSUM"))
        ps_sc = ctx.enter_context(tc.tile_pool(name="ps_sc", bufs=2, space="PSUM"))
        ps_pt = ctx.enter_context(tc.tile_pool(name="ps_pt", bufs=1, space="PSUM"))
        ps_u = ctx.enter_context(tc.tile_pool(name="ps_u", bufs=1, space="PSUM"))

        def load_chunk(c):
            xt_ = xpool.tile([128, NJ, H], BF16, tag="x")
            src_ = X[c * TC : (c + 1) * TC, :].rearrange("(j p) h -> p j h", p=128)
            nc.gpsimd.dma_start(out=xt_, in_=src_)
            return xt_

        def load_chunk0_split():
            xt_ = xpool.tile([128, NJ, H], BF16, tag="x")
            src_ = X[0:TC, :].rearrange("(j p) h -> p j h", p=128)
            for j in range(NJ):
                nc.gpsimd.dma_start(out=xt_[:, j, :], in_=src_[:, j, :])
            return xt_

        x_tiles = [load_chunk0_split()]

        ident = singles.tile([128, 128], BF16)
        nc.sync.dma_start(out=ident, in_=IDENT)
        aux_sb = singles.tile([1, 128 + T], BF16)
        nc.sync.dma_start(out=aux_sb, in_=AUX)
        ones_row = aux_sb[:, 0:NH]
        mT_sb = singles.tile([128, KC, NH], BF16)
        nc.sync.dma_start(out=mT_sb, in_=MT.rearrange("(c p) n -> p c n", p=128))

        d_sb = singles.tile([NH, NC_], F32)
        u_ps = ps_u.tile([NH, H], F32)
        e_tiles = {}

        x_tiles.append(load_chunk(1))

        # PE warm-up: dummy matmuls on the identity while chunk-0 DMA lands.
        # Keeps the HAM activity monitor busy so real matmuls start at 2.4GHz.
        warm_ps = ps_pt.tile([128, 128], F32, tag="pt")
        for _ in range(56):
            nc.tensor.matmul(warm_ps, ident, ident, start=True, stop=True)

        def emit_tail(c):
            # probsT + u-accumulation for chunk c
            x_c = x_tiles[c]
            e_sb = e_tiles[c]
            pT_ps = ps_pt.tile([128, NJ, NH], BF16, tag="pt")
            for j in range(NJ):
                nc.tensor.transpose(
                    pT_ps[:, j, :],
                    e_sb[:, j * 128 : (j + 1) * 128],
                    ident[0:NH, 0:NH],
                )
            pT_sb = ptpool.tile([128, NJ, NH], BF16, tag="pts")
            nc.vector.tensor_copy(pT_sb, pT_ps)
            for j in range(NJ):
                for n in range(2):
                    nc.tensor.matmul(
                        u_ps[:, n * 512 : (n + 1) * 512],
                        pT_sb[:, j, :],
                        x_c[:, j, n * 512 : (n + 1) * 512],
                        start=(c == 0 and j == 0),
                        stop=(c == NC_ - 1 and j == NJ - 1),
                    )

        for c in range(NC_):
            if c + 2 < NC_:
                x_tiles.append(load_chunk(c + 2))
            x_c = x_tiles[c]

            sc_ps = ps_sc.tile([NH, TC], F32, tag="sc")
            xT_done = []
            for kc2 in range(0, KC, 2):
                for kc in (kc2, kc2 + 1):
                    xT_ps = ps_xt.tile([128, TC], BF16, tag="xt")
                    for j in range(NJ):
                        nc.tensor.transpose(
                            xT_ps[:, j * 128 : (j + 1) * 128],
                            x_c[:, j, kc * 128 : (kc + 1) * 128],
                            ident,
                        )
                    xT_sb = xtpool.tile([128, TC], BF16, tag="xts")
                    nc.vector.tensor_copy(xT_sb, xT_ps)
                    xT_done.append(xT_sb)
                if kc2 == 0:
                    # open the accumulation group with the mask rank-1 add
                    # (no copy dependency - fills the first bubble)
                    nc.tensor.matmul(
                        sc_ps, ones_row,
                        aux_sb[:, 128 + c * TC : 128 + (c + 1) * TC],
                        start=True, stop=False,
                    )
                if kc2 >= 2:
                    # paired mm1s, lag one pair: copy latency hidden
                    nc.tensor.matmul(
                        sc_ps, mT_sb[:, kc2 - 2, :], xT_done[kc2 - 2],
                        start=False, stop=False,
                    )
                    nc.tensor.matmul(
                        sc_ps, mT_sb[:, kc2 - 1, :], xT_done[kc2 - 1],
                        start=False, stop=False,
                    )
                if kc2 == 4 and c >= 1:
                    emit_tail(c - 1)
            nc.tensor.matmul(
                sc_ps, mT_sb[:, KC - 2, :], xT_done[KC - 2],
                start=False, stop=False,
            )
            nc.tensor.matmul(
                sc_ps, mT_sb[:, KC - 1, :], xT_done[KC - 1],
                start=False, stop=True,
            )

            e_sb = epool.tile([NH, TC], BF16, tag="e")
            e_tiles[c] = e_sb
            nc.scalar.activation(
                out=e_sb, in_=sc_ps,
                func=mybir.ActivationFunctionType.Exp,
                accum_out=d_sb[:, c : c + 1],
            )
        nc.sync.dma_start(out=D, in_=d_sb)
        emit_tail(NC_ - 1)

        u_sb = singles.tile([NH, H], F32)
        for n in range(2):
            nc.vector.tensor_copy(
                u_sb[:, n * 512 : (n + 1) * 512], u_ps[:, n * 512 : (n + 1) * 512]
            )
            nc.sync.dma_start(
                out=U[:, n * 512 : (n + 1) * 512],
                in_=u_sb[:, n * 512 : (n + 1) * 512],
            )

    nc.compile()
    return nc


def _build_f32r():
    import concourse.tile as tile
    from concourse import bacc, mybir
    from contextlib import ExitStack

    F32 = mybir.dt.float32
    F32R = mybir.dt.float32r

    nc = bacc.Bacc("TRN2", target_bir_lowering=False, debug=False)
    X = nc.dram_tensor("X", [T, H], F32, kind="ExternalInput").ap()
    MT = nc.dram_tensor("MT", [H, 128], F32, kind="ExternalInput").ap()
    AUX = nc.dram_tensor("AUX", [1, 128 + T], F32, kind="ExternalInput").ap()
    IDENT = nc.dram_tensor("IDENT", [128, 128], F32, kind="ExternalInput").ap()
    U = nc.dram_tensor("U", [NH, H], F32, kind="ExternalOutput").ap()
    D = nc.dram_tensor("D", [NH, NCH], F32, kind="ExternalOutput").ap()

    with tile.TileContext(nc) as tc, ExitStack() as ctx:
        singles = ctx.enter_context(tc.tile_pool(name="singles", bufs=1))
        xpool = ctx.enter_context(tc.tile_pool(name="xpool", bufs=5))
        stpool = ctx.enter_context(tc.tile_pool(name="stpool", bufs=4))
        xtpool = ctx.enter_context(tc.tile_pool(name="xtpool", bufs=3))
        epool = ctx.enter_context(tc.tile_pool(name="epool", bufs=2))
        ps_xt = ctx.enter_context(tc.tile_pool(name="ps_xt", bufs=2, space="PSUM"))
        ps_sc = ctx.enter_context(tc.tile_pool(name="ps_sc", bufs=2, space="PSUM"))
        ps_pt = ctx.enter_context(tc.tile_pool(name="ps_pt", bufs=2, space="PSUM"))
        ps_u = ctx.enter_context(tc.tile_pool(name="ps_u", bufs=1, space="PSUM"))

        ident = singles.tile([128, 128], F32R)
        nc.gpsimd.dma_start(out=ident, in_=IDENT)
        aux_sb = singles.tile([1, 128 + T], F32R)
        nc.gpsimd.dma_start(out=aux_sb, in_=AUX)
        ones_row = aux_sb[:, 0:128]
        mT_sb = singles.tile([128, H // 128, 128], F32R)
        nc.gpsimd.dma_start(out=mT_sb, in_=MT.rearrange("(c p) n -> p c n", p=128))

        d_sb = singles.tile([NH, NCH], F32)
        u_ps = ps_u.tile([128, H], F32)

        pT_hold = []
        for i in range(2):
            t_ = singles.tile([128, 4, 128], F32R, tag=f"pT{i}")
            nc.vector.memset(t_.bitcast(F32), 0.0)
            pT_hold.append(t_)

        for c in range(NCH):
            x_c = xpool.tile([128, 4, H], F32R, tag="x")
            nc.gpsimd.dma_start(
                out=x_c,
                in_=X[c * TCH : (c + 1) * TCH, :].rearrange(
                    "(j p) h -> p j h", p=128
                ),
            )

            sc_ps = ps_sc.tile([128, TCH], F32, tag="sc")
            for kc in range(H // 128):
                xT_ps = ps_xt.tile([128, TCH], F32, tag="xt")
                for j in range(4):
                    nc.tensor.transpose(
                        xT_ps.bitcast(F32R)[:, j * 128 : (j + 1) * 128],
                        x_c[:, j, kc * 128 : (kc + 1) * 128],
                        ident,
                    )
                xT_sb = xtpool.tile([128, TCH], F32R, tag="xts")
                if kc % 2 == 0:
                    nc.vector.tensor_copy(xT_sb, xT_ps)
                else:
                    nc.scalar.copy(xT_sb, xT_ps)
                nc.tensor.matmul(
                    sc_ps, mT_sb[:, kc, :], xT_sb,
                    start=(kc == 0), stop=False,
                )
            nc.tensor.matmul(
                sc_ps, ones_row,
                aux_sb[:, 128 + c * TCH : 128 + (c + 1) * TCH],
                start=False, stop=True,
            )

            e_sb = epool.tile([NH, TCH], F32R, tag="e")
            nc.scalar.activation(
                out=e_sb, in_=sc_ps[0:NH, :],
                func=mybir.ActivationFunctionType.Exp,
                accum_out=d_sb[:, c : c + 1],
            )

            pT_ps = ps_pt.tile([128, 4, NH], F32, tag="pt")
            for j in range(4):
                nc.tensor.transpose(
                    pT_ps.bitcast(F32R)[:, j, :],
                    e_sb[:, j * 128 : (j + 1) * 128],
                    ident[0:NH, 0:NH],
                )
            hold = pT_hold[c % 2]
            nc.vector.tensor_copy(hold[:, :, 0:NH], pT_ps)

            for j in range(4):
                for n in range(2):
                    nc.tensor.matmul(
                        u_ps[:, n * TCH : (n + 1) * TCH],
                        hold[:, j, :],
                        x_c[:, j, n * TCH : (n + 1) * TCH],
                        start=(c == 0 and j == 0),
                        stop=(c == NCH - 1 and j == 3),
                    )

        u_sb = singles.tile([NH, H], F32)
        nc.vector.tensor_copy(u_sb, u_ps[0:NH, :])
        nc.sync.dma_start(out=U, in_=u_sb)
        nc.sync.dma_start(out=D, in_=d_sb)

    nc.compile()
    return nc


def kernel(
    hidden_states, cell_state, Wq, bq, Wk, bk, Wv, bv, attention_mask, t
):
    global LAST_RESULTS
    import ml_dtypes
    from concourse.bass_utils import run_bass_kernel_spmd

    hs = np.asarray(hidden_states, dtype=np.float32)
    cell = np.asarray(cell_state, dtype=np.float32)
    Wq_ = np.asarray(Wq, dtype=np.float32)
    bq_ = np.asarray(bq, dtype=np.float32)
    Wk_ = np.asarray(Wk, dtype=np.float32)
    bk_ = np.asarray(bk, dtype=np.float32)  # noqa: F841 (drops out of softmax)
    Wv_ = np.asarray(Wv, dtype=np.float32)
    bv_ = np.asarray(bv, dtype=np.float32)
    mask = np.asarray(attention_mask)
    tt = int(t)
    assert tt == T and hs.shape == (B, T + 1, H)

    # host precompute: q, m (fold the 1/sqrt(hd) scale into m)
    q = (cell @ Wq_.T + bq_).reshape(B, NH, HD)
    Wk_h = Wk_.reshape(NH, HD, H)
    m = np.einsum("bhd,hdi->bhi", q, Wk_h) / np.sqrt(HD)  # [B, NH, H]

    maskadd = np.where(mask[:, :T] == 0, MASK_NEG, 0.0).astype(np.float32)

    if USE_BF16:
        bf16 = ml_dtypes.bfloat16
        mT = np.ascontiguousarray(
            m.transpose(0, 2, 1)
        ).astype(bf16)  # [B, H, NH]
        aux = np.zeros((B, 1, 128 + T), dtype=bf16)
        aux[:, 0, :NH] = bf16(1.0)
        aux[:, 0, 128:] = maskadd.astype(bf16)
        eye = np.eye(128, dtype=np.float32).astype(bf16)
    else:
        mT = np.zeros((B, H, 128), dtype=np.float32)
        mT[:, :, :NH] = m.transpose(0, 2, 1)
        aux = np.zeros((B, 1, 128 + T), dtype=np.float32)
        aux[:, 0, :128] = 1.0
        aux[:, 0, 128:] = maskadd
        eye = np.eye(128, dtype=np.float32)

    key = "bf16" if USE_BF16 else "f32r"
    if key not in _prog_cache:
        _prog_cache[key] = _build_bf16() if USE_BF16 else _build_f32r()
    nc = _prog_cache[key]

    in_maps = [
        {
            "X": np.ascontiguousarray(hs[b, :T, :]),
            "MT": mT[b],
            "AUX": aux[b],
            "IDENT": eye,
        }
        for b in range(B)
    ]
    res = run_bass_kernel_spmd(nc, in_maps, core_ids=list(range(B)))
    LAST_RESULTS = res

    U = np.stack(
        [np.asarray(res.results[b]["U"], dtype=np.float32) for b in range(B)]
    )
    Dp = np.stack(
        [np.asarray(res.results[b]["D"], dtype=np.float32) for b in range(B)]
    )
    denom = Dp.sum(axis=2)  # [B, NH]
    u = U / denom[:, :, None]
    Wv_h = Wv_.reshape(NH, HD, H)
    ctx = np.einsum("bhi,hdi->bhd", u, Wv_h) + bv_.reshape(NH, HD)[None]
    return ctx.reshape(B, NH * HD).astype(np.float32)
